# revision 18
# baseline (speedup 1.0000x reference)
"""Single-head attention (B=2, S=2048, D=2048, fp32 in/out) on 8 trn2 cores.

Sharding: sequence-parallel. 4096 tokens (B*S) split 512/core; cores 0-3 hold
batch 0, cores 4-7 batch 1. Each core projects K^T and V for its 512 tokens,
the shards are AllGathered within each 4-core group, then each core computes
scores -> softmax -> attn@V -> @W_o for its 512 queries against the full 2048
keys of its batch.

All matmul operands are bf16 (fp32 PSUM accumulation). bf16 enables the PE's
fast-weight-load path so the per-matmul LDWEIGHTS is hidden behind the N=512
stream, and it halves DMA + collective traffic vs fp32r.

Per-core phases (256 matmuls of K=128, M=128, N=512 each):
  B: KT[e,t]   = mm(lhsT=Wk[d,e],   rhs=xt[d,t])     -> DRAM, AllGather
  C: V[t,e]    = mm(lhsT=xt[d,t],   rhs=Wv[d,e])     -> DRAM, AllGather
  D: QT[e,q]   = mm(lhsT=Wq[d,e],   rhs=xt[d,q])     -> SBUF
  E: sT[k,q]   = mm(lhsT=KTg[e,k],  rhs=QT[e,q]); exp(s*scale) -> attnT bf16
     (scoresT layout: no transposes; 1/sqrt(D) folded into the exp scale)
     rowsum[1,q] += mm(lhsT=ones[k,1], rhs=attnT[k,q])  (16 extra matmuls)
  F: outT[e,q] = mm(lhsT=Vg[k,e],   rhs=attnT[k,q])  -> SBUF
  G: final[q,d]= mm(lhsT=outT[e,q], rhs=Wo[e,d]) * (1/rowsum) -> out DRAM

Weight/V streams load as full (128, 2048) rows (4KB DMA lines); K^T-gather
streams as (128, 512) tiles.
"""
import math
import numpy as np
import ml_dtypes

import concourse.bass as bass
import concourse.mybir as mybir
import concourse.tile as tile
from concourse import bacc
from concourse.bass_utils import run_bass_kernel_spmd

F32 = mybir.dt.float32
BF16 = mybir.dt.bfloat16

D = 2048          # d_model
B = 2
S = 2048
NCORES = 8
GS = 4            # cores per batch group
TOK = 512         # tokens (queries) per core
P = 128
NT = D // P       # 16 tiles along d/e
SCALE = 1.0 / math.sqrt(D)


def build_attn(n_iters=1, with_collective=True, psum_bufs=7, w_bufs=20,
               v_bufs=20, k_bufs=24, unroll=False):
    """Build the SPMD attention kernel. n_iters>1 wraps all compute phases in
    a timing loop: a For_i hardware loop by default (collectives must be off —
    they desync the mesh inside For_i), or unrolled python bodies when
    unroll=True (collectives allowed)."""
    nc = bacc.Bacc("TRN2", target_bir_lowering=False, debug=False, num_devices=NCORES)

    xt = nc.dram_tensor("xt", [D, TOK], BF16, kind="ExternalInput")
    wq = nc.dram_tensor("wq", [D, D], BF16, kind="ExternalInput")
    wk = nc.dram_tensor("wk", [D, D], BF16, kind="ExternalInput")
    wv = nc.dram_tensor("wv", [D, D], BF16, kind="ExternalInput")
    wo = nc.dram_tensor("wo", [D, D], BF16, kind="ExternalInput")
    out = nc.dram_tensor("out", [TOK, D], F32, kind="ExternalOutput")

    with tile.TileContext(nc) as tc:
        with (
            tc.tile_pool(name="dram", bufs=1, space="DRAM") as dram,
            tc.tile_pool(name="big", bufs=1) as big,
            tc.tile_pool(name="stream", bufs=w_bufs) as stream,
            tc.tile_pool(name="kstream", bufs=k_bufs) as kstream,
            tc.tile_pool(name="qtpool", bufs=NT) as qtpool,
            tc.tile_pool(name="evpool", bufs=4) as evpool,
            tc.tile_pool(name="misc", bufs=1) as misc,
            tc.tile_pool(name="ps", bufs=psum_bufs, space="PSUM") as ps,
        ):
            kt_shard = dram.tile([D, TOK], BF16)
            v_shard = [dram.tile([TOK, D // 2], BF16, name=f"v_shard{h}")
                       for h in range(2)]
            kt_g = [dram.tile([GS * (D // 2), TOK], BF16, name=f"kt_g{h}")
                    for h in range(2)]
            v_g = [dram.tile([GS * TOK, D // 2], BF16, name=f"v_g{h}")
                   for h in range(2)]
            rsum_d = dram.tile([1, TOK], F32)

            xt_sb = [big.tile([P, TOK], BF16, name=f"xt{i}") for i in range(NT)]
            attnT_sb = [big.tile([P, TOK], BF16, name=f"attnT{i}") for i in range(NT)]

            ones_bf = misc.tile([P, 1], BF16)
            nc.gpsimd.memset(ones_bf, 1.0)
            rstage = misc.tile([P, 4], F32)
            recip = misc.tile([P, 4], F32)
            sums_sb = misc.tile([1, TOK], F32)

            _eng_i = [0]
            _engines = [nc.sync, nc.scalar]

            def LD(dst, src_ap):
                e = _engines[_eng_i[0] % len(_engines)]
                _eng_i[0] += 1
                e.dma_start(dst, src_ap)

            # ---- load x^T into SBUF (once; outside the timing loop)
            for dt in range(NT):
                nc.sync.dma_start(xt_sb[dt][:], xt[dt * P:(dt + 1) * P, :])

            def stream_rows(pfx, src, rows=NT):
                """Load full-width (128, 2048) rows of a DRAM matrix."""
                ts = []
                for i in range(rows):
                    t = stream.tile([P, D], BF16, tag="stream", name=f"{pfx}{i}")
                    LD(t[:], src(i))
                    ts.append(t)
                return ts

            def proj_to_T(w_dram, dest_cb, pfx, hook=None):
                """KT/QT projection: out[e,t] = sum_d W[d,e]*xt[d,t]."""
                wts = stream_rows(pfx, lambda dt: w_dram[dt * P:(dt + 1) * P, :])
                for eg in range(4):
                    psums = [ps.tile([P, TOK], F32, tag="mm", name=f"{pfx}p{i}")
                             for i in range(4)]
                    for half in range(2):
                        for j in range(4):
                            for dt8 in range(8):
                                dt = half * 8 + dt8
                                nc.tensor.matmul(
                                    psums[j][:],
                                    wts[dt][:, eg * 512 + j * P: eg * 512 + (j + 1) * P],
                                    xt_sb[dt][:],
                                    start=(dt == 0), stop=(dt == NT - 1))
                    for j in range(4):
                        dest_cb(eg * 4 + j, psums[j])
                    if hook is not None:
                        hook(eg)

            def b_dest(et, psum):
                ev = evpool.tile([P, TOK], BF16, tag="ev", name="evb")
                nc.vector.tensor_copy(ev[:], psum[:])
                nc.sync.dma_start(kt_shard[et * P:(et + 1) * P, :], ev[:])

            def phase_c(hook=None):
                wvs = stream_rows("cw", lambda dt: wv[dt * P:(dt + 1) * P, :])
                for ec in range(4):
                    psums = [ps.tile([P, TOK], F32, tag="mm", name=f"pvp{i}")
                             for i in range(4)]
                    for half in range(2):
                        for tt in range(4):
                            for dt8 in range(8):
                                dt = half * 8 + dt8
                                nc.tensor.matmul(
                                    psums[tt][:],
                                    xt_sb[dt][:, tt * P:(tt + 1) * P],
                                    wvs[dt][:, ec * 512:(ec + 1) * 512],
                                    start=(dt == 0), stop=(dt == NT - 1))
                    for tt in range(4):
                        ev = evpool.tile([P, TOK], BF16, tag="ev", name="evc")
                        nc.scalar.copy(ev[:], psums[tt][:])
                        nc.sync.dma_start(
                            v_shard[ec // 2][tt * P:(tt + 1) * P,
                                             (ec % 2) * 512:(ec % 2 + 1) * 512],
                            ev[:])
                    if hook is not None:
                        hook(ec)

            def phases_defg():
                # ---- phase D: QT (tiles share slots with outT via tag)
                qt_sb = [qtpool.tile([P, TOK], BF16, tag="qo", name=f"qt{i}")
                         for i in range(NT)]

                def d_dest(et, psum):
                    nc.vector.tensor_copy(qt_sb[et][:], psum[:])
                proj_to_T(wq, d_dest, "pd")

                # ---- phase E: scoresT + exp (k-partitioned; no transposes)
                for s in range(GS):
                    kts = []
                    for et in range(NT):
                        t = kstream.tile([P, TOK], BF16, tag="ks", name=f"ek{et}")
                        src = kt_g[et // 8]
                        LD(t[:], src[s * (D // 2) + (et % 8) * P:
                                     s * (D // 2) + (et % 8 + 1) * P, :])
                        kts.append(t)
                    psums = [ps.tile([P, TOK], F32, tag="mm", name=f"pep{i}")
                             for i in range(4)]
                    for half in range(2):
                        for ksub in range(4):
                            for et8 in range(8):
                                et = half * 8 + et8
                                nc.tensor.matmul(
                                    psums[ksub][:],
                                    kts[et][:, ksub * P:(ksub + 1) * P],
                                    qt_sb[et][:],
                                    start=(et == 0), stop=(et == NT - 1))
                    for ksub in range(4):
                        nc.scalar.activation(
                            attnT_sb[s * 4 + ksub][:], psums[ksub][:],
                            mybir.ActivationFunctionType.Exp, scale=SCALE)

                # ---- softmax row sums: rowsum[1,q] = sum_k attnT[k,q]
                psum_r = ps.tile([1, TOK], F32, tag="rs", bufs=1, name="psum_r")
                for kt in range(NT):
                    nc.tensor.matmul(
                        psum_r[:], ones_bf[:], attnT_sb[kt][:],
                        start=(kt == 0), stop=(kt == NT - 1))
                nc.scalar.copy(sums_sb[:], psum_r[:])
                nc.sync.dma_start(rsum_d[:, :], sums_sb[:])
                for qt in range(4):
                    nc.sync.dma_start(rstage[:, qt:qt + 1],
                                      rsum_d[0:1, qt * P:(qt + 1) * P])
                nc.vector.reciprocal(recip[:], rstage[:])

                # ---- phase F: outT[e,q] (slots freed by qt after phase E)
                outT_sb = [qtpool.tile([P, TOK], BF16, tag="qo", name=f"outT{i}")
                           for i in range(NT)]
                vts = [[None, None] for _ in range(NT)]
                for h in range(2):
                    for kt in range(NT):
                        t = stream.tile([P, D // 2], BF16, tag="vstream",
                                        bufs=v_bufs, name=f"fv{kt}_{h}")
                        LD(t[:], v_g[h][kt * P:(kt + 1) * P, :])
                        vts[kt][h] = t
                for eg in range(4):
                    psums = [ps.tile([P, TOK], F32, tag="mm", name=f"pfp{i}")
                             for i in range(4)]
                    for half in range(2):
                        for j in range(4):
                            for kt8 in range(8):
                                kt = half * 8 + kt8
                                nc.tensor.matmul(
                                    psums[j][:],
                                    vts[kt][eg // 2][:, (eg % 2) * 512 + j * P:
                                                     (eg % 2) * 512 + (j + 1) * P],
                                    attnT_sb[kt][:],
                                    start=(kt == 0), stop=(kt == NT - 1))
                    for j in range(4):
                        nc.scalar.copy(outT_sb[eg * 4 + j][:], psums[j][:])

                # ---- phase G: final[q,d] = outT^T @ Wo, scaled by 1/rowsum
                wos = stream_rows("gw", lambda et: wo[et * P:(et + 1) * P, :])
                for dc in range(4):
                    psums = [ps.tile([P, TOK], F32, tag="mm", name=f"pgp{i}")
                             for i in range(4)]
                    for half in range(2):
                        for qt in range(4):
                            for et8 in range(8):
                                et = half * 8 + et8
                                nc.tensor.matmul(
                                    psums[qt][:],
                                    outT_sb[et][:, qt * P:(qt + 1) * P],
                                    wos[et][:, dc * 512:(dc + 1) * 512],
                                    start=(et == 0), stop=(et == NT - 1))
                    for qt in range(4):
                        evf = evpool.tile([P, TOK], F32, tag="evf", name="evf")
                        nc.vector.tensor_scalar_mul(evf[:], psums[qt][:],
                                                    recip[:, qt:qt + 1])
                        nc.sync.dma_start(
                            out[qt * P:(qt + 1) * P, dc * 512:(dc + 1) * 512], evf[:])

            def gather(in_ap, out_ap):
                nc.gpsimd.collective_compute(
                    "AllGather", mybir.AluOpType.bypass,
                    replica_groups=[[0, 1, 2, 3], [4, 5, 6, 7]],
                    ins=[in_ap.opt()], outs=[out_ap.opt()],
                )

            def whole_body():
                def b_hook(eg):
                    if with_collective in (True, "k") and eg in (1, 3):
                        h = eg // 2
                        gather(kt_shard[h * (D // 2):(h + 1) * (D // 2), :],
                               kt_g[h][:])
                proj_to_T(wk, b_dest, "pb", hook=b_hook)

                def c_hook(ec):
                    if with_collective in (True, "v") and ec in (1, 3):
                        h = ec // 2
                        gather(v_shard[h][:], v_g[h][:])
                phase_c(hook=c_hook)
                phases_defg()

            if n_iters == 1:
                whole_body()
            elif unroll:
                for _ in range(n_iters):
                    whole_body()
            else:
                assert not with_collective, "collectives desync inside For_i"
                with tc.For_i(0, n_iters, 1):
                    whole_body()

    nc.compile()
    return nc


_CACHED = {}


def _get_nc():
    if "nc" not in _CACHED:
        _CACHED["nc"] = build_attn()
    return _CACHED["nc"]


def _make_in_maps(inputs):
    bf16 = ml_dtypes.bfloat16
    x = np.asarray(inputs["x"], np.float32)
    wq_c = np.ascontiguousarray(np.asarray(inputs["W_q"], np.float32).astype(bf16))
    wk_c = np.ascontiguousarray(np.asarray(inputs["W_k"], np.float32).astype(bf16))
    wv_c = np.ascontiguousarray(np.asarray(inputs["W_v"], np.float32).astype(bf16))
    wo_c = np.ascontiguousarray(np.asarray(inputs["W_o"], np.float32).astype(bf16))

    toks = x.reshape(B * S, D)              # (4096, 2048)
    xt_full = np.ascontiguousarray(toks.T).astype(bf16)  # (2048, 4096)

    in_maps = []
    for c in range(NCORES):
        in_maps.append({
            "xt": np.ascontiguousarray(xt_full[:, c * TOK:(c + 1) * TOK]),
            "wq": wq_c, "wk": wk_c, "wv": wv_c, "wo": wo_c,
        })
    return in_maps


def kernel(x, W_q, W_k, W_v, W_o):
    in_maps = _make_in_maps(dict(x=x, W_q=W_q, W_k=W_k, W_v=W_v, W_o=W_o))
    nc = _get_nc()
    res = run_bass_kernel_spmd(nc, in_maps, core_ids=list(range(NCORES)))
    rows = np.concatenate([res.results[c]["out"] for c in range(NCORES)], axis=0)
    return rows.reshape(B, S, D)
